# revision 1
# baseline (speedup 1.0000x reference)
"""Multi-head causal attention (B=2, S=2048, D=1024, H=16) on 8 Trainium2
NeuronCores.

Sharding: tensor-parallel over heads - 2 heads per core. Each core computes
its heads' Q/K/V projections, causal attention, and a partial output
projection (row-parallel over the head dims); the host sums the 8 partials
and adds the output bias.

v2: bf16 device pipeline (fp32 PSUM accumulation, ~5e-3 rel err), merged
DMAs (whole X^T resident in SBUF via paired-row 3-D DRAM access patterns,
two DMAs per output q-block; bulk X loads go through Pool's SWDGE so the
two HWDGE queues stay free for startup-critical weight loads), exact causal
offsets, per-tag PSUM bank rings, and a DVE diet (single strided V-pack
copy, 64-wide parallel reciprocal, PSUM evacuation split DVE/ACT - ACT
Copy/Identity share the Exp act table so they never cost a table switch;
ACT helps only where it has slack: batch-0 attention and the flush tail).

Device layout is fully "transposed" (features on partitions, tokens on the
free axis):
  - QKV projection:  QKVT[f, t]  via lhsT=W^T tiles, rhs=X^T tiles; QKV
    token-blocks stream through the whole kernel (attention q-block g only
    needs token-blocks <= g, so tb_{g+1} projects inside q-block g's
    attention stream, filling the PE's exp-wait gaps)
  - V is immediately PE-transposed to token-major and packed with 64 ones
    columns per (k-tile, head) block into `vaug`; Q^T/K^T stay feature-major
  - scores^T[k, q] = KT_tile^T @ QT_block   (contraction = head dim 64)
  - softmax along k (= partitions): exp on ACT, causal mask via bf16
    mask-mult (DVE 2x mode); the denominator rides the attn@V matmul
    through vaug's ones columns, landing replicated on ps_o partitions
    64..127 so the reciprocal runs 64-partition-parallel straight into
    SBUF (no PE broadcast, no extra PSUM bank)
  - attn^T[hd, q] accumulates over k tiles in PSUM f32
  - out^T[e, t] partial = WoutT^T @ attnT, summed across cores on the host
"""

import contextlib

import numpy as np
import ml_dtypes

import bass_rust
import concourse.bass as bass
import concourse.mybir as mybir
from concourse.bass_utils import run_bass_kernel_spmd
from concourse.tile import TileContext
from concourse.masks import make_identity

F32 = mybir.dt.float32
F32R = mybir.dt.float32r
BF16 = mybir.dt.bfloat16

B, S, D, H = 2, 2048, 1024, 16
HD = D // H            # 64
NCORES = 8
HPC = H // NCORES      # heads per core = 2
DSL = HPC * HD         # feature slice per core = 128
T = B * S              # 4096 tokens
NT = T // 512          # 8 token blocks of 512
ND = D // 128          # 8 d-tiles
NKT = S // 128         # 16 k-tiles per batch
NQB = S // 512         # 4 q-blocks per batch


def _split_multi_waits(nc):
    """This walrus build accepts only ONE sync-wait per instruction. Hoist
    all-but-one wait of any multi-wait instruction onto same-engine NoOps
    placed immediately before it (engine program order preserves
    semantics)."""
    n = 0
    for f in nc.m.functions:
        for blk in f.blocks:
            il = blk.instructions
            new = []
            changed = False
            for inst in il:
                si = inst.sync_info
                waits = list(si.on_wait) if si is not None and si.on_wait else []
                if len(waits) > 1:
                    changed = True
                    for w in waits[:-1]:
                        nop = mybir.InstNoOp(
                            name=f"I-waitsplit-{nc.next_id()}", ins=[], outs=[]
                        )
                        nop.engine = inst.engine
                        nop.sync_info = bass_rust.SyncInfo(on_wait=[w], on_update=[])
                        new.append(nop)
                        n += 1
                    inst.sync_info = bass_rust.SyncInfo(
                        on_wait=[waits[-1]], on_update=list(si.on_update or [])
                    )
                new.append(inst)
            if changed:
                blk.instructions = new
    return n


def _build(loop_n=None, loop_phase=None):
    import os as _os

    # PSUM bank budget (8 banks of [128, 512] f32): per-tag ring depths
    PS_O = int(_os.environ.get("K_PS_O", "2"))      # attn@V accumulators
    PS_S = int(_os.environ.get("K_PS_S", "2"))      # score tiles
    PS_Q = int(_os.environ.get("K_PS_Q", "2"))      # qkv projection chains
    PS_OUT = int(_os.environ.get("K_PS_OUT", "2"))  # out-projection tiles
    PS_T = int(_os.environ.get("K_PS_T", "1"))      # V transpose staging

    nc = bass.Bass("TRN2", target_bir_lowering=False, debug=False)

    XT = nc.declare_dram_parameter("XT", [D, T], BF16, isOutput=False)
    WQKVT = nc.declare_dram_parameter("WQKVT", [D, 3 * DSL], BF16, isOutput=False)
    BQKV = nc.declare_dram_parameter("BQKV", [DSL, 3], F32, isOutput=False)
    WOUTT = nc.declare_dram_parameter("WOUTT", [DSL, D], BF16, isOutput=False)
    OUTT = nc.declare_dram_parameter("OUTT", [D, T], BF16, isOutput=True)

    EXP = mybir.ActivationFunctionType.Exp
    scale = 1.0 / np.sqrt(HD)

    with TileContext(nc) as tc:
        with (
            tc.tile_pool(name="const", bufs=1) as const,
            tc.tile_pool(name="big", bufs=1) as big,
            tc.tile_pool(name="ep", bufs=int(_os.environ.get("K_EP", "10"))) as ep,
            tc.tile_pool(name="e2p", bufs=int(_os.environ.get("K_E2P", "8"))) as e2p,
            tc.tile_pool(name="vsb", bufs=int(_os.environ.get("K_VSB", "3"))) as vsbp,
            tc.tile_pool(name="work", bufs=int(_os.environ.get("K_WORK", "4"))) as work,
            tc.tile_pool(name="obp", bufs=int(_os.environ.get("K_OBP", "6"))) as obp,
            tc.tile_pool(name="accps", bufs=1, space="PSUM") as accps,
            tc.tile_pool(name="strps", bufs=1, space="PSUM") as strps,
            contextlib.ExitStack() as _loop_ctx,
        ):
            def phase_loop():
                if loop_n is not None:
                    stag = _os.environ.get("K_STAGGER", "0") == "1"
                    return tc.For_i(0, loop_n, 1, staggered_reset=stag)
                return contextlib.nullcontext()

            # ---- constants (outside any bench loop) ---------------------
            ident_f = const.tile([128, 128], F32, name="ident_f")
            make_identity(nc, ident_f)
            ident = const.tile([128, 128], BF16, name="ident")
            nc.vector.tensor_copy(ident, ident_f)
            # diagonal causal-mask tiles: mask[j][kk, qq] = (qq - kk - 128j >= 0)
            masks = []
            for j in range(4):
                mk = const.tile([128, 512], BF16, name=f"mask{j}")
                nc.gpsimd.memset(mk, 1.0)
                nc.gpsimd.affine_select(
                    out=mk, in_=mk, compare_op=mybir.AluOpType.is_ge,
                    fill=0.0, base=-128 * j, channel_multiplier=-1,
                    pattern=[[1, 512]],
                )
                masks.append(mk)

            # ---- persistent activations ---------------------------------
            # whole X^T resident, d-tiles merged in pairs to halve DMA
            # dispatch count: first superblock (4 tiles of [128, 2x1024]),
            # rest (4 tiles of [128, 2x3072])
            xtq = [big.tile([128, 2048], BF16, name=f"xtq{i}") for i in range(4)]
            xtr = [big.tile([128, 6144], BF16, name=f"xtr{i}") for i in range(4)]
            # Q^T, K^T feature-major (two heads stacked on partitions)
            qkvt = [big.tile([128, T], BF16, name=f"qkvt{f}") for f in range(2)]
            # V token-major + 64 ones columns per (k-tile, head) block of
            # 128: the attn@V matmul then lands the softmax denominator
            # replicated on ps_o partitions 64..127, so the reciprocal runs
            # 64-partition-parallel and no PE broadcast is needed.
            nblk = (T // 128) * 2
            vaug = big.tile([128, nblk * 2 * HD], BF16, name="vaug")
            attnt = big.tile([128, T], BF16, name="attnt")
            # QKV weights f-major: wqf[f] cols = d*128 (one DMA per f)
            wqf = [const.tile([128, ND * DSL], BF16, name=f"wqf{f}")
                   for f in range(3)]
            woutt = const.tile([DSL, D], BF16, name="woutt")
            bqkv = const.tile([DSL, 3], F32, name="bqkv")
            # all ones-columns (cols 64..127 of each 128 block) in one memset
            nc.gpsimd.memset(
                vaug.rearrange("p (b c) -> p b c", c=2 * HD)[:, :, HD:2 * HD],
                1.0,
            )

            def emit_input_dmas():
                # ordering tuned for startup: f=0 weights + first-superblock
                # X land first so the first QKV chain starts ~3us in
                qs = [nc.sync, nc.scalar]
                nc.sync.dma_start(
                    out=wqf[0].rearrange("p (b c) -> p b c", c=DSL),
                    in_=WQKVT[:, 0:DSL].rearrange("(b p) c -> p b c", b=ND),
                )
                wq_eng = (nc.gpsimd if loop_n is None and _os.environ.get(
                    "K_WQ_SWDGE", "0") == "1" else nc.scalar)
                wq_eng.dma_start(
                    out=wqf[1].rearrange("p (b c) -> p b c", c=DSL),
                    in_=WQKVT[:, DSL:2 * DSL].rearrange(
                        "(b p) c -> p b c", b=ND),
                )
                _xp = int(_os.environ.get("K_XQ_PAR", "0"))
                for i in range(4):
                    qs[(i + _xp) % 2].dma_start(
                        out=xtq[i].rearrange("p (b c) -> p b c", b=2),
                        in_=XT[i * 256:(i + 1) * 256, 0:1024].rearrange(
                            "(b p) c -> p b c", b=2),
                    )
                wq_eng.dma_start(
                    out=wqf[2].rearrange("p (b c) -> p b c", c=DSL),
                    in_=WQKVT[:, 2 * DSL:3 * DSL].rearrange(
                        "(b p) c -> p b c", b=ND),
                )
                nc.scalar.dma_start(out=bqkv, in_=BQKV[:, :])
                # bulk X loads go through Pool's SWDGE in one-shot mode: a
                # third dispatch path that keeps HWDGE free for the
                # startup-critical weight/first-superblock DMAs (SWDGE
                # inside For_i is avoided, mirroring the v1 kernel)
                xq = (nc.gpsimd if loop_n is None or _os.environ.get(
                    "K_SWDGE_LOOP", "0") == "1" else None)
                for i in range(4):
                    eng = xq if xq is not None else qs[i % 2]
                    eng.dma_start(
                        out=xtr[i].rearrange("p (b c) -> p b c", b=2),
                        in_=XT[i * 256:(i + 1) * 256, 1024:T].rearrange(
                            "(b p) c -> p b c", b=2),
                    )
                (nc.gpsimd if loop_n is None else nc.scalar).dma_start(
                    out=woutt, in_=WOUTT[:, :])

            # loop_phase=0: input DMAs hoisted out of the bench loop
            # (diagnostic for loop-mode timing); default keeps them in-loop
            if loop_n is not None and loop_phase == 0:
                emit_input_dmas()

            with phase_loop():
                if not (loop_n is not None and loop_phase == 0):
                    emit_input_dmas()

                # ---- work-item emitters ---------------------------------
                def emit_qkv_group(sb2, th, f):
                    """One [128, 512] projection tile: 8 matmuls + bias add.
                    For V (f == 2) also transpose to token-major into vaug."""
                    t = sb2 * 2 + th
                    ps = strps.tile([128, 512], F32, name="ps_qkv", tag="q",
                                    bufs=PS_Q)
                    for d in range(ND):
                        if sb2 == 0:
                            col = (d % 2) * 1024 + th * 512
                            x_ap = xtq[d // 2][:, col:col + 512]
                        else:
                            col = ((d % 2) * 3 + sb2 - 1) * 1024 + th * 512
                            x_ap = xtr[d // 2][:, col:col + 512]
                        nc.tensor.matmul(
                            ps,
                            wqf[f][:, d * DSL:(d + 1) * DSL],
                            x_ap,
                            start=(d == 0),
                            stop=(d == ND - 1),
                        )
                    bias_act = sb2 < int(_os.environ.get("K_BIAS_ACT", "0"))
                    if f < 2:
                        dst_q = qkvt[f][:, t * 512:(t + 1) * 512]
                        if bias_act:
                            nc.scalar.add(dst_q, ps, bqkv[:, f:f + 1])
                        else:
                            nc.vector.tensor_scalar_add(dst_q, ps,
                                                        bqkv[:, f:f + 1])
                        return
                    vsb = vsbp.tile([128, 512], BF16, name="vsb", tag="v")
                    if bias_act:
                        nc.scalar.add(vsb, ps, bqkv[:, 2:3])
                    else:
                        nc.vector.tensor_scalar_add(vsb, ps, bqkv[:, 2:3])
                    # 4 PE transposes into one tile, then ONE strided copy
                    # into vaug (src blocks of 64 -> dst blocks of 64 spaced
                    # 65 apart; ones columns already set)
                    tp4 = strps.tile([128, 512], BF16, name="tp4", tag="po",
                                     bufs=PS_OUT)
                    for i in range(4):
                        nc.tensor.transpose(
                            tp4[:, i * 128:(i + 1) * 128],
                            vsb[:, i * 128:(i + 1) * 128], ident,
                        )
                    base = t * 8 * 2 * HD
                    dst = vaug[:, base:base + 16 * HD].rearrange(
                        "p (b c) -> p b c", c=2 * HD)[:, :, 0:HD]
                    src = tp4.rearrange("p (b c) -> p b c", c=HD)
                    nc.vector.tensor_copy(dst, src)

                def emit_scores(b, qb, kt, h):
                    """scores^T + exp (+ causal mask on diagonal tiles).
                    Returns (tile, column offset) for attn@V. For diagonal
                    tile j = kt - 4*qb, query columns < 128*j are entirely
                    masked, so everything runs on the [off:512] slice."""
                    qc = b * S + qb * 512
                    ktg = b * NKT + kt
                    off = 0
                    diag = kt >= 4 * qb
                    if diag:
                        off = min(128 * (kt - 4 * qb), 384)
                    ps_s = strps.tile([128, 512], F32, name="ps_s", tag="s",
                                      bufs=PS_S)
                    nc.tensor.matmul(
                        ps_s[:, off:512],
                        qkvt[1][h * HD:(h + 1) * HD, ktg * 128:(ktg + 1) * 128],
                        qkvt[0][h * HD:(h + 1) * HD, qc + off:qc + 512],
                        start=True, stop=True, tile_position=(h * HD, 0),
                    )
                    expt = ep.tile([128, 512], BF16, name="expt", tag="e")
                    nc.scalar.activation(
                        expt[:, off:512], ps_s[:, off:512], EXP, scale=scale
                    )
                    if diag:
                        # all-SBUF bf16 multiply: DVE 2x perf mode
                        expt2 = e2p.tile([128, 512], BF16, name="expt2", tag="e2")
                        nc.vector.tensor_mul(
                            expt2[:, off:512], expt[:, off:512],
                            masks[kt - 4 * qb][:, off:512],
                        )
                        return expt2, off
                    return expt, off

                def emit_attnv(ps_o, b, qb, kt, h, src_off, nkt):
                    src_tile, off = src_off
                    ktg = b * NKT + kt
                    va = vaug[:, (ktg * 2 + h) * 2 * HD:
                              (ktg * 2 + h + 1) * 2 * HD]
                    nc.tensor.matmul(
                        ps_o[h][:, off:512], va, src_tile[:, off:512],
                        start=(kt == 0), stop=(kt == nkt - 1),
                    )

                def make_epilogue(ps_o, b, qb):
                    def epi():
                        qc = b * S + qb * 512
                        for h in range(2):
                            # denominator sits replicated on partitions
                            # 64..127 of ps_o; 64-wide parallel reciprocal
                            # into SBUF, then one dual-input multiply
                            recip = work.tile([HD, 512], F32, name="recip",
                                              tag="r", bufs=2)
                            with nc.allow_low_precision(reason="softmax denom"):
                                nc.vector.reciprocal(
                                    recip, ps_o[h][HD:2 * HD, :])
                            nc.vector.tensor_mul(
                                attnt[h * HD:(h + 1) * HD, qc:qc + 512],
                                ps_o[h][0:HD, :], recip,
                            )
                    return epi

                in_tail = [False]  # True once all exps are emitted (flush)
                evac_ctr = [0]

                def make_outproj(b, qb):
                    def opj():
                        tb = b * S + qb * 512
                        ob = obp.tile([128, ND * 512], BF16, name="ob",
                                      tag="ob")
                        for e in range(ND):
                            ps = strps.tile([128, 512], F32, name="ps_out",
                                            tag="po", bufs=PS_OUT)
                            nc.tensor.matmul(
                                ps,
                                woutt[:, e * 128:(e + 1) * 128],
                                attnt[:, tb:tb + 512],
                                start=True, stop=True,
                            )
                            # split PSUM evacuation DVE/ACT (Pool cannot
                            # read PSUM; ACT Copy shares the exp act-table).
                            # ACT only helps where it has slack: 1 in 4
                            # during batch-0 attention, half in the tail.
                            k = evac_ctr[0] = evac_ctr[0] + 1
                            dst = ob[:, e * 512:(e + 1) * 512]
                            if in_tail[0]:
                                m = int(_os.environ.get("K_EVAC_TAIL", "2"))
                            elif b == 0:
                                m = int(_os.environ.get("K_EVAC_ACT", "2"))
                            else:
                                m = int(_os.environ.get("K_EVAC_B1", "0"))
                            use_act = m > 0 and k % m == int(
                                _os.environ.get("K_EVAC_PH", "0")) % m
                            if use_act:
                                nc.scalar.copy(dst, ps)
                            else:
                                nc.vector.tensor_copy(dst, ps)
                        # two DMAs per q-block (halves of the e range):
                        # the first overlaps the remaining evacuations
                        nh = ND // 2
                        oeng = (nc.gpsimd
                                if loop_n is None and _os.environ.get(
                                    "K_OUT_SWDGE", "0") == "1"
                                else nc.sync)
                        for half in range(2):
                            oeng.dma_start(
                                out=OUTT[half * 512:(half + 1) * 512,
                                         tb:tb + 512].rearrange(
                                    "(b p) c -> p b c", b=nh),
                                in_=ob[:, half * nh * 512:
                                       (half + 1) * nh * 512].rearrange(
                                    "p (b c) -> p b c", c=512),
                            )
                    return opj

                # ---- interleaved emission -------------------------------
                # Attention q-block g only depends on QKV token-blocks <= g,
                # so QKV streams through the whole kernel: tb0 is the only
                # serial prefix; tb_{g+1}'s three projection groups thread
                # into q-block g's attention stream (filling the PE's
                # exp-wait gaps) along with the previous q-block's epilogue
                # and out-projection.
                qkv_items = []
                pending = []
                tail_outproj = []

                def pop_item():
                    # epilogues first: releases ps_o accumulator banks sooner
                    if pending:
                        pending.pop(0)()
                    elif qkv_items:
                        qkv_items.pop(0)()

                for f in range(3):
                    emit_qkv_group(0, 0, f)

                # process q-blocks [0,1,3,2] per batch so the last block of
                # each batch is 12 k-tiles, not 16 (the 16-k-tile block is
                # ACT-heaviest; running it 3rd hides its exps better).
                # enq[i] = token-blocks whose QKV groups are threaded into
                # processing step i (must all be emitted before any step
                # that reads them).
                _steps_env = _os.environ.get("K_STEPS", "")
                if _steps_env:
                    steps = [(int(s[0]), int(s[1]))
                             for s in _steps_env.split(",")]
                else:
                    _order = _os.environ.get("K_ORDER", "0123")
                    _qbs = [int(c) for c in _order]
                    steps = [(b, qb) for b in range(B) for qb in _qbs]
                # token-block g is first needed by the step that processes
                # q-block g; thread its QKV groups into the previous step
                need_at = {}
                for i, (b, qb) in enumerate(steps):
                    gg = b * NQB + qb
                    for tb in range(gg + 1):
                        if tb not in need_at:
                            need_at[tb] = i
                enq = {}
                for tb, i in sorted(need_at.items()):
                    if i > 0:
                        enq.setdefault(i - 1, []).append(tb)
                for i, (b, qb) in enumerate(steps):
                        for tb in enq.get(i, []):
                            for f in range(3):
                                qkv_items.append(
                                    (lambda tb=tb, f=f:
                                     emit_qkv_group(tb // 2, tb % 2, f))
                                )
                        nkt = 4 * qb + 4
                        ps_o = [
                            accps.tile([128, 512], F32,
                                       name=f"ps_o{h}", tag="o", bufs=PS_O)
                            for h in range(2)
                        ]
                        # lookahead: scores/exp run DEPTH k-tiles ahead of
                        # the attn@V consuming them, covering the exp+mask
                        # latency with matmul groups.
                        DEPTH = int(__import__("os").environ.get("K_DEPTH", "4"))
                        window = []
                        for kt0 in range(min(DEPTH, nkt)):
                            window.append(
                                [emit_scores(b, qb, kt0, h) for h in range(2)]
                            )
                            pop_item()
                        for kt in range(DEPTH, nkt):
                            cur = [emit_scores(b, qb, kt, h) for h in range(2)]
                            pop_item()
                            old_srcs = window.pop(0)
                            for h in range(2):
                                emit_attnv(ps_o, b, qb, kt - DEPTH, h,
                                           old_srcs[h], nkt)
                            window.append(cur)
                        base = max(0, nkt - DEPTH)
                        for j, srcs in enumerate(window):
                            for h in range(2):
                                emit_attnv(ps_o, b, qb, base + j, h,
                                           srcs[h], nkt)
                        pending.append(make_epilogue(ps_o, b, qb))
                        # batch-1 out-projections are deferred to the flush
                        # tail: their PE work then overlaps the ACT exp
                        # drain instead of competing with the PE-saturated
                        # mid-kernel (epilogues still pop promptly to free
                        # the ps_o accumulator banks)
                        if b == 1 and _os.environ.get("K_DEFER_B1", "0") == "1":
                            tail_outproj.append(make_outproj(b, qb))
                        else:
                            pending.append(make_outproj(b, qb))
                for fn in qkv_items:
                    fn()
                in_tail[0] = True
                for fn in pending:
                    fn()
                for fn in tail_outproj:
                    fn()

    nc.finalize()
    _split_multi_waits(nc)
    return nc


_NC = None
LAST_EXEC_TIME_NS = None


def _prep_in_maps(X, W_qkv, b_qkv, W_out, b_out):
    X = np.asarray(X, dtype=np.float32)
    W_qkv = np.asarray(W_qkv, dtype=np.float32)
    b_qkv = np.asarray(b_qkv, dtype=np.float32)
    W_out = np.asarray(W_out, dtype=np.float32)

    XTv = np.ascontiguousarray(X.reshape(T, D).T.astype(ml_dtypes.bfloat16))

    in_maps = []
    for c in range(NCORES):
        sl = slice(c * DSL, (c + 1) * DSL)
        wc = np.concatenate(
            [W_qkv[c * DSL:(c + 1) * DSL],
             W_qkv[D + c * DSL:D + (c + 1) * DSL],
             W_qkv[2 * D + c * DSL:2 * D + (c + 1) * DSL]],
            axis=0,
        )
        wqkvt = np.ascontiguousarray(wc.T.astype(ml_dtypes.bfloat16))
        bq = np.stack(
            [b_qkv[sl], b_qkv[D + sl.start:D + sl.stop],
             b_qkv[2 * D + sl.start:2 * D + sl.stop]],
            axis=1,
        )
        woutt = np.ascontiguousarray(
            W_out[:, sl].T.astype(ml_dtypes.bfloat16))
        in_maps.append(
            {
                "XT": XTv,
                "WQKVT": wqkvt,
                "BQKV": np.ascontiguousarray(bq.astype(np.float32)),
                "WOUTT": woutt,
            }
        )
    return in_maps


def kernel(X, W_qkv, b_qkv, W_out, b_out):
    global _NC, LAST_EXEC_TIME_NS
    b_out = np.asarray(b_out, dtype=np.float32)
    in_maps = _prep_in_maps(X, W_qkv, b_qkv, W_out, b_out)

    if _NC is None:
        _NC = _build()
    res = run_bass_kernel_spmd(_NC, in_maps, core_ids=list(range(NCORES)))
    LAST_EXEC_TIME_NS = res.exec_time_ns

    total = res.results[0]["OUTT"].astype(np.float64)
    for r in res.results[1:]:
        total += r["OUTT"].astype(np.float64)
    out = total.T + b_out
    return np.ascontiguousarray(out.reshape(B, S, D).astype(np.float32))

